# revision 13
# baseline (speedup 1.0000x reference)
"""APPNP GNN kernel for 8 Trainium2 NeuronCores.

Strategy:
  - Row-partition nodes across 8 cores (12500 nodes/core, padded to 12800).
  - MLP computed per-core on the tensor engine (feature-major matmuls,
    weights pre-transposed on host), transposed back to node-major via PE.
  - 10 power iterations. Each iteration:
      * AllGather the propagated features V (bf16, padded to 256B rows)
        into a replicated DRAM table [102400, 128]bf16.
      * dma_gather (custom SWDGE gather) pulls neighbor rows in
        128-edge chunks. Chunks are grouped by (dst-block, src-quartile);
        quartile split is forced by the gather's int16 index range.
      * Segment-reduce per 128-dst block via one-hot matmuls on the PE:
        lhsT = R (one-hot of dst-local id, built by a DVE is_equal against
        an iota constant), rhs = gathered rows -> PSUM accumulate.
      * Per-block epilogue: add self-loop row (V local shard), scale by
        c1*deg_inv (per-partition scalar on ACT), add c2*h.
  - Output: node-major fp32 [12500, 40] per core, unpermuted on host.

Destination nodes are relabeled per-core by descending degree so the
per-(block, quartile) chunk counts are nearly equal across cores (the
program structure is shared by all 8 cores).
"""

import math

import numpy as np
import ml_dtypes

import concourse.bass as bass
import concourse.bacc as bacc
import concourse.tile as tile
import concourse.mybir as mybir
from concourse import bass_utils
from concourse.masks import make_identity

F32 = mybir.dt.float32
BF16 = mybir.dt.bfloat16
I16 = mybir.dt.int16

N_CORES = 8
P = 128
NFEAT, NHID, NCLASS = 512, 256, 40
C1 = 0.5  # ALPHA/(1+ALPHA), ALPHA=1.0
C2 = 0.5  # 1/(1+ALPHA)
EW = 128  # table row width in bf16 (256 bytes)
NQ = 4  # quartile count (int16 index range)
MAXC = 8  # max gather chunks per dma_gather call (1024-idx ucode cap)
PAD_DLOC = 200.0  # sentinel dst-local id for pad slots (one-hot row = 0)


# ---------------------------------------------------------------- host prep
def preprocess(x, edge_index, n_powers):
    x = np.asarray(x, np.float32)
    n = x.shape[0]
    shard = n // N_CORES
    assert shard * N_CORES == n
    nloc = ((shard + P - 1) // P) * P
    blocks = nloc // P
    quart = (N_CORES * nloc) // NQ
    assert quart <= 32768, "int16 gather index range exceeded"

    dst = np.asarray(edge_index[0]).astype(np.int64)
    src = np.asarray(edge_index[1]).astype(np.int64)

    deg = np.bincount(dst, minlength=n).astype(np.float64) + 1.0
    c1deginv = (C1 / deg).astype(np.float32)

    # per-core degree-descending relabeling
    order = np.empty((N_CORES, shard), np.int64)  # rank -> local id
    rank_of = np.empty(n, np.int64)  # global node -> rank within its core
    prow = np.empty(n, np.int64)  # global node -> permuted table row
    for c in range(N_CORES):
        d = deg[c * shard:(c + 1) * shard]
        o = np.argsort(-d, kind="stable")
        order[c] = o
        rank_of[c * shard + o] = np.arange(shard)
        prow[c * shard + o] = c * nloc + np.arange(shard)

    e_core = dst // shard
    e_rank = rank_of[dst]
    e_b = e_rank // P
    e_dloc = e_rank % P
    e_prow = prow[src]
    e_q = e_prow // quart
    e_ridx = e_prow - e_q * quart

    # per-core per-(block, q) edge counts -> shared chunk structure
    cnt = np.zeros((N_CORES, blocks, NQ), np.int64)
    np.add.at(cnt, (e_core, e_b, e_q), 1)
    C = np.ceil(cnt.max(axis=0) / P).astype(np.int64)  # [blocks, NQ]

    colstart = np.zeros((blocks, NQ), np.int64)
    totc = np.zeros(NQ, np.int64)
    for qq in range(NQ):
        colstart[:, qq] = np.cumsum(C[:, qq]) - C[:, qq]
        totc[qq] = C[:, qq].sum()
    qoff = np.concatenate([[0], np.cumsum(totc)])  # column offset of q stream
    tot_cols = int(qoff[-1])

    # greedy block grouping: per group, per q, sum of C <= MAXC
    groups = []
    b0 = 0
    while b0 < blocks:
        b1 = b0 + 1
        while b1 < blocks and all(C[b0:b1 + 1, qq].sum() <= MAXC
                                  for qq in range(NQ)):
            b1 += 1
        groups.append((b0, b1))
        b0 = b1

    # fill per-core slot arrays
    idxw = np.zeros((N_CORES, P, tot_cols * 8), np.int16)
    dloc_arr = np.full((N_CORES, P, tot_cols), PAD_DLOC, ml_dtypes.bfloat16)

    # sort edges by (core, q, block, ridx)
    skey = np.lexsort((e_ridx, e_b, e_q, e_core))
    sc, sb, sq = e_core[skey], e_b[skey], e_q[skey]
    sridx, sdloc = e_ridx[skey], e_dloc[skey]
    # position of each edge within its (core, q, b) bucket; key must be
    # monotone in the lexsort order (core, q, b) so unique() indices ascend
    bucket = ((sc * NQ + sq) * blocks + sb)
    uniq, first_pos = np.unique(bucket, return_index=True)
    pos_in_bucket = np.arange(len(skey)) - np.repeat(
        first_pos, np.diff(np.concatenate([first_pos, [len(skey)]])))

    col = qoff[sq] + colstart[sb, sq] + pos_in_bucket // P
    slot = pos_in_bucket % P
    # idx wrapped layout: element j=(col_rel*128+slot) of a call starting at
    # col c0 lives at [ (j%16) + 16*g for g ], free = col*8 + (slot//16)... but
    # absolute: flatpos = col*128 + slot (within q stream, absolute cols work
    # because calls slice columns; relative position preserved).
    part16 = slot % 16
    free = col * 8 + slot // 16
    for g in range(8):
        idxw[sc, part16 + 16 * g, free] = sridx
    dloc_arr[sc, slot, col] = sdloc.astype(np.float32)

    # per-core dense tensors
    xt = np.zeros((N_CORES, NFEAT, nloc), np.float32)
    dg = np.zeros((N_CORES, P, blocks), np.float32)
    for c in range(N_CORES):
        ids = c * shard + order[c]
        xt[c, :, :shard] = x[ids].T
        dgv = np.zeros(nloc, np.float32)
        dgv[:shard] = c1deginv[ids]
        dg[c] = dgv.reshape(blocks, P).T

    iota = np.broadcast_to(
        np.arange(P, dtype=np.float32)[None, None, :], (P, MAXC, P))
    iota = np.ascontiguousarray(iota.reshape(P, MAXC * P)).astype(
        ml_dtypes.bfloat16)

    struct = dict(n=n, shard=shard, nloc=nloc, blocks=blocks, quart=quart,
                  C=C, colstart=colstart, qoff=qoff[:NQ], tot_cols=tot_cols,
                  groups=groups, n_powers=n_powers)
    percore = dict(idxw=idxw, dloc=dloc_arr, xt=xt, dg=dg)
    shared = dict(iota=iota)
    return struct, percore, shared, order


# ------------------------------------------------------------- bass program
def build_program(st, bench_iters=None, skip=()):
    nloc, blocks, quart = st["nloc"], st["blocks"], st["quart"]
    tot_cols = st["tot_cols"]
    C, colstart, qoff, groups = st["C"], st["colstart"], st["qoff"], st["groups"]
    n_powers = st["n_powers"]
    ntab = N_CORES * nloc
    tcol = max(t for t in (512, 384, 256, 128) if nloc % t == 0)
    ntiles = nloc // tcol
    kf, kh = NFEAT // P, NHID // P

    nc = bacc.Bacc("TRN2", target_bir_lowering=False, debug=False,
                   enable_asserts=False, num_devices=N_CORES,
                   num_swdge_queues=4)

    xt_in = nc.dram_tensor("xt", [NFEAT, nloc], F32, kind="ExternalInput")
    w1t_in = nc.dram_tensor("w1t", [NFEAT, NHID], F32, kind="ExternalInput")
    w2t_in = nc.dram_tensor("w2t", [NHID, NHID], F32, kind="ExternalInput")
    w3t_in = nc.dram_tensor("w3t", [NHID, NCLASS], F32, kind="ExternalInput")
    b1_in = nc.dram_tensor("b1c", [P, 2], F32, kind="ExternalInput")
    b2_in = nc.dram_tensor("b2c", [P, 2], F32, kind="ExternalInput")
    b3_in = nc.dram_tensor("b3c", [NCLASS, 1], F32, kind="ExternalInput")
    idx_in = nc.dram_tensor("idxw", [P, tot_cols * 8], I16, kind="ExternalInput")
    dloc_in = nc.dram_tensor("dloc", [P, tot_cols], BF16, kind="ExternalInput")
    dg_in = nc.dram_tensor("dg", [P, blocks], F32, kind="ExternalInput")
    iota_in = nc.dram_tensor("iota", [P, MAXC * P], BF16, kind="ExternalInput")
    out_t = nc.dram_tensor("out", [nloc, NCLASS], F32, kind="ExternalOutput")

    with tile.TileContext(nc) as tc, \
            tc.tile_pool(name="dramp", bufs=1, space="DRAM") as dp, \
            tc.tile_pool(name="persist", bufs=1) as pp:
        # persistent DRAM
        ag_in_t = dp.tile([nloc, EW], BF16, name="ag_in")
        ntables = 1 if bench_iters is not None else n_powers
        tables = [dp.tile([ntab, EW], BF16, addr_space="Shared",
                          name=f"table{i}") for i in range(ntables)]

        # persistent SBUF state
        idx_sb = pp.tile([P, tot_cols * 8], I16, name="idx_sb")
        dloc_sb = pp.tile([P, tot_cols], BF16, name="dloc_sb")
        dg_sb = pp.tile([P, blocks], F32, name="dg_sb")
        iota_sb = pp.tile([P, MAXC, P], BF16, name="iota_sb")
        h_sb = pp.tile([P, blocks, NCLASS], F32, name="h_sb")  # = V (fp32)
        c2h_sb = pp.tile([P, blocks, NCLASS], F32, name="c2h_sb")
        vout_sb = pp.tile([P, blocks, EW], BF16, name="vout_sb")

        nc.sync.dma_start(out=idx_sb[:], in_=idx_in.ap())
        nc.sync.dma_start(out=dloc_sb[:], in_=dloc_in.ap())
        nc.sync.dma_start(out=dg_sb[:], in_=dg_in.ap())
        nc.sync.dma_start(out=iota_sb[:].rearrange("p a b -> p (a b)"),
                          in_=iota_in.ap())
        nc.vector.memset(vout_sb[:], 0.0)

        # ---------------- MLP ----------------
        with tc.tile_pool(name="mw", bufs=1) as mw, \
                tc.tile_pool(name="mact", bufs=2) as mact, \
                tc.tile_pool(name="mps", bufs=1, space="PSUM") as mps, \
                tc.tile_pool(name="mps2", bufs=2, space="PSUM") as mps2:
            ident = mw.tile([P, P], F32)
            make_identity(nc, ident[:])
            w1_sb = mw.tile([P, kf, NHID], F32)
            nc.sync.dma_start(
                out=w1_sb[:],
                in_=w1t_in.ap().rearrange("(a p) m -> p a m", p=P))
            w2_sb = mw.tile([P, kh, NHID], F32)
            nc.sync.dma_start(
                out=w2_sb[:],
                in_=w2t_in.ap().rearrange("(a p) m -> p a m", p=P))
            w3_sb = mw.tile([P, kh, NCLASS], F32)
            nc.sync.dma_start(
                out=w3_sb[:],
                in_=w3t_in.ap().rearrange("(a p) m -> p a m", p=P))
            b1_sb = mw.tile([P, 2], F32)
            nc.sync.dma_start(out=b1_sb[:], in_=b1_in.ap())
            b2_sb = mw.tile([P, 2], F32)
            nc.sync.dma_start(out=b2_sb[:], in_=b2_in.ap())
            b3_sb = mw.tile([NCLASS, 1], F32)
            nc.sync.dma_start(out=b3_sb[:], in_=b3_in.ap())

            xt_r = xt_in.ap().rearrange("(a p) t -> p a t", p=P)
            for t in range(ntiles):
                sl = slice(t * tcol, (t + 1) * tcol)
                xtile = mact.tile([P, kf, tcol], F32, tag="xt")
                nc.sync.dma_start(out=xtile[:], in_=xt_r[:, :, sl])
                h1p = mps.tile([P, 2, tcol], F32, tag="h1p")
                for m in range(2):
                    for k in range(kf):
                        nc.tensor.matmul(
                            out=h1p[:, m, :],
                            lhsT=w1_sb[:, k, m * P:(m + 1) * P],
                            rhs=xtile[:, k, :],
                            start=(k == 0), stop=(k == kf - 1))
                h1 = mact.tile([P, 2, tcol], F32, tag="h1")
                for m in range(2):
                    nc.scalar.activation(
                        h1[:, m, :], h1p[:, m, :],
                        mybir.ActivationFunctionType.Relu,
                        bias=b1_sb[:, m:m + 1])
                h2p = mps.tile([P, 2, tcol], F32, tag="h2p")
                for m in range(2):
                    for k in range(kh):
                        nc.tensor.matmul(
                            out=h2p[:, m, :],
                            lhsT=w2_sb[:, k, m * P:(m + 1) * P],
                            rhs=h1[:, k, :],
                            start=(k == 0), stop=(k == kh - 1))
                h2 = mact.tile([P, 2, tcol], F32, tag="h2")
                for m in range(2):
                    nc.scalar.activation(
                        h2[:, m, :], h2p[:, m, :],
                        mybir.ActivationFunctionType.Relu,
                        bias=b2_sb[:, m:m + 1])
                h3p = mps2.tile([P, tcol], F32, tag="h3p")
                for k in range(kh):
                    nc.tensor.matmul(
                        out=h3p[:NCLASS, :],
                        lhsT=w3_sb[:, k, :],
                        rhs=h2[:, k, :],
                        start=(k == 0), stop=(k == kh - 1))
                h3 = mact.tile([NCLASS, tcol], F32, tag="h3")
                nc.vector.tensor_tensor(
                    out=h3[:], in0=h3p[:NCLASS, :],
                    in1=b3_sb[:].to_broadcast([NCLASS, tcol]),
                    op=mybir.AluOpType.add)
                for i in range(tcol // P):
                    trp = mps2.tile([P, NCLASS], F32, tag="trp")
                    nc.tensor.transpose(
                        out=trp[:], in_=h3[:, i * P:(i + 1) * P],
                        identity=ident[:NCLASS, :NCLASS])
                    nc.scalar.activation(
                        h_sb[:, t * (tcol // P) + i, :], trp[:],
                        mybir.ActivationFunctionType.Copy)

        nc.scalar.activation(c2h_sb[:], h_sb[:],
                             mybir.ActivationFunctionType.Copy, scale=C2)
        nc.scalar.activation(vout_sb[:, :, :NCLASS], h_sb[:],
                             mybir.ActivationFunctionType.Copy)

        # ---------------- power iterations ----------------
        ag_dst = ag_in_t[:].rearrange("(b p) e -> p b e", p=P)
        rg = [list(range(N_CORES))]
        totc = [int(C[:, q].sum()) for q in range(NQ)]
        with tc.tile_pool(name="gp", bufs=10) as gp, \
                tc.tile_pool(name="rp", bufs=10) as rp, \
                tc.tile_pool(name="yp", bufs=4, space="PSUM") as yp, \
                tc.tile_pool(name="ep", bufs=4) as ep:
            def emit_iter(table_t, with_ag=True):
                nc.sync.dma_start(out=ag_dst, in_=vout_sb[:])
                if with_ag:
                    nc.gpsimd.collective_compute(
                        "AllGather", mybir.AluOpType.bypass,
                        replica_groups=rg,
                        ins=[ag_in_t[:]], outs=[table_t[:]])
                emitted = [0] * NQ
                wtiles = {}
                for b in range(blocks):
                    for q in range(NQ):
                        cb = int(C[b, q])
                        if cb == 0:
                            continue
                        c0 = int(colstart[b, q])
                        w_hi = (c0 + cb - 1) // MAXC
                        for w in range(emitted[q], w_hi + 1):
                            cc = min(MAXC, totc[q] - w * MAXC)
                            gc = int(qoff[q]) + w * MAXC
                            g = gp.tile([P, MAXC, EW], BF16, tag="g")
                            gcc = 1 if "gsmall" in skip else cc
                            if "gather" not in skip:
                              nc.gpsimd.dma_gather(
                                out_ap=g[:, :gcc, :],
                                in_ap=table_t[q * quart:, :],
                                idxs_ap=idx_sb[:, gc * 8:(gc + gcc) * 8],
                                num_idxs=gcc * P,
                                num_idxs_reg=gcc * P,
                                elem_size=EW,
                                queue_num=q,
                            )
                            r = rp.tile([P, MAXC, P], BF16, tag="r")
                            rcc = 1 if "rsmall" in skip else cc
                            if "rgen" not in skip:
                              nc.vector.tensor_tensor(
                                out=r[:, :rcc, :],
                                in0=dloc_sb[:, gc:gc + rcc].unsqueeze(-1)
                                    .to_broadcast([P, rcc, P]),
                                in1=iota_sb[:, :rcc, :],
                                op=mybir.AluOpType.is_equal)
                            wtiles[(q, w)] = (g, r)
                        emitted[q] = w_hi + 1
                    mms = []
                    if "matmul" not in skip:
                      for q in range(NQ):
                        cb = int(C[b, q])
                        c0 = int(colstart[b, q])
                        for k in range(cb):
                            col = c0 + k
                            g, r = wtiles[(q, col // MAXC)]
                            lc = col % MAXC
                            mms.append((r[:, lc, :], g[:, lc, :NCLASS]))
                    ypt = yp.tile([P, NCLASS], F32, tag="y")
                    if "msmall" in skip:
                        mms = mms[:1]
                    for j, (lhs, rhs) in enumerate(mms):
                        nc.tensor.matmul(
                            out=ypt[:], lhsT=lhs, rhs=rhs,
                            start=(j == 0), stop=(j == len(mms) - 1))
                    t1 = ep.tile([P, NCLASS], F32, tag="t1")
                    if mms:
                        nc.vector.tensor_tensor(
                            out=t1[:], in0=ypt[:], in1=h_sb[:, b, :],
                            op=mybir.AluOpType.add)
                    else:
                        nc.vector.tensor_copy(t1[:], h_sb[:, b, :])
                    t2 = ep.tile([P, NCLASS], F32, tag="t2")
                    nc.scalar.activation(
                        t2[:], t1[:], mybir.ActivationFunctionType.Copy,
                        scale=dg_sb[:, b:b + 1])
                    nc.vector.tensor_tensor(
                        out=h_sb[:, b, :], in0=t2[:], in1=c2h_sb[:, b, :],
                        op=mybir.AluOpType.add)
                    nc.scalar.activation(
                        vout_sb[:, b, :NCLASS], h_sb[:, b, :],
                        mybir.ActivationFunctionType.Copy)

            if bench_iters is None:
                for it in range(n_powers):
                    emit_iter(tables[it])
            else:
                emit_iter(tables[0])
                with tc.For_i(0, bench_iters, 1):
                    emit_iter(tables[0], with_ag=False)

        nc.sync.dma_start(
            out=out_t.ap().rearrange("(b p) d -> p b d", p=P),
            in_=h_sb[:])
    nc.compile()
    return nc


# ------------------------------------------------------------------- driver
def _run(x, edge_index, W1, b1, W2, b2, W3, b3, n_powers,
         bench_iters=None, skip=()):
    st, pc, sh, order = preprocess(x, edge_index, n_powers)
    nc = build_program(st, bench_iters=bench_iters, skip=skip)
    w1t = np.ascontiguousarray(np.asarray(W1, np.float32).T)
    w2t = np.ascontiguousarray(np.asarray(W2, np.float32).T)
    w3t = np.ascontiguousarray(np.asarray(W3, np.float32).T)
    b1c = np.ascontiguousarray(np.asarray(b1, np.float32).reshape(2, P).T)
    b2c = np.ascontiguousarray(np.asarray(b2, np.float32).reshape(2, P).T)
    b3c = np.asarray(b3, np.float32).reshape(NCLASS, 1)
    in_maps = []
    for c in range(N_CORES):
        in_maps.append({
            "xt": np.ascontiguousarray(pc["xt"][c]),
            "w1t": w1t, "w2t": w2t, "w3t": w3t,
            "b1c": b1c, "b2c": b2c, "b3c": b3c,
            "idxw": np.ascontiguousarray(pc["idxw"][c]),
            "dloc": np.ascontiguousarray(pc["dloc"][c]),
            "dg": np.ascontiguousarray(pc["dg"][c]),
            "iota": sh["iota"],
        })
    res = bass_utils.run_bass_kernel_spmd(nc, in_maps,
                                          core_ids=list(range(N_CORES)))
    n, shard = st["n"], st["shard"]
    out = np.empty((n, NCLASS), np.float32)
    for c in range(N_CORES):
        out[c * shard + order[c]] = res.results[c]["out"][:shard]
    return out


def kernel(x, edge_index, W1, b1, W2, b2, W3, b3):
    return _run(x, edge_index, W1, b1, W2, b2, W3, b3, n_powers=10)
